# revision 20
# baseline (speedup 1.0000x reference)
"""GCN layer (dropout -> weighted segment-sum aggregation -> linear -> ReLU)
on 8 Trainium2 NeuronCores.

Strategy (matches the "shard nodes by destination" hint):
  - Destination nodes (output rows) are sharded 6250/core across 8 cores.
  - Edges are partitioned by destination (edge_dst is sorted), then grouped
    into 128-destination segments per core; each segment's edges are chunked
    into groups of 128.
  - Source features are gathered per-edge from a replicated (dropout-applied,
    bf16) copy of x via the GPSIMD dma_gather custom op (int16 indices, so x
    is addressed through two 25000-row bank views).
  - Weighted segment-sum runs on the tensor engine: for each 128-edge chunk a
    selection matrix S[e, d] = w_e * (dst_e == d) is built on the vector
    engine (iota == rel) * w and matmul'd against the gathered rows,
    accumulating agg[d, f] for the segment in PSUM.
  - agg is transposed on the PE (via identity), multiplied by W (+ bias via a
    ones-row matmul), ReLU'd on the scalar engine, and DMA'd out.

All 8 cores run the same program (SPMD, one NEFF); per-core data differences
(edge partitions) are carried entirely by the input tensors. Per-(segment,
bank) chunk counts are padded to the max across cores with zero-weight edges.

The dropout mask depends only on the fixed key(42) and the input shape, so it
is constant-folded into the replicated feature table on the host.
"""

import os

import numpy as np
import ml_dtypes

# Problem constants (hardcoded per the harness contract).
N_NODES = 50000
N_EDGES = 800000
F = 512
P = 128
N_CORES = 8
RPC = N_NODES // N_CORES          # 6250 rows (dst nodes) per core
SEG = 128                          # dst nodes per segment
NSEG = (RPC + SEG - 1) // SEG      # 49 (last segment has 106 rows)
BANK = 25000                       # int16-addressable row banks of x
KEEP = 0.9                         # 1 - dropout p

LAST_RESULTS = None                # BassKernelResults of the last run (for profiling)
_PROGRAM_CACHE = {}


def _ensure_axon_ntff_hook():
    """bass_utils imports antenv.axon_hooks when BASS_TRACE=1 under axon;
    some images lack that module. Provide it (with the ctypes NTFF hook from
    trn_agent_boot when available) so tracing works instead of crashing."""
    import sys
    import types

    try:
        import antenv.axon_hooks  # noqa: F401
        return
    except ImportError:
        pass
    try:
        import antenv
    except ImportError:
        return
    hook = None
    try:
        from trn_agent_boot.trn_boot import _ntff_profile_via_ctypes

        hook = _ntff_profile_via_ctypes("/opt/axon/libaxon_pjrt.so")
    except Exception:
        hook = None
    mod = types.ModuleType("antenv.axon_hooks")
    _state = {"hook": hook}
    mod.get_axon_ntff_profile_hook = lambda: _state["hook"]
    mod.set_axon_ntff_profile_hook = lambda h: _state.__setitem__("hook", h)
    sys.modules["antenv.axon_hooks"] = mod
    antenv.axon_hooks = mod


def _dropout_table(x):
    """Apply the deterministic dropout (jax key 42) and cast to bf16."""
    import jax

    cpu = jax.devices("cpu")[0]
    with jax.default_device(cpu):
        keep = np.asarray(
            jax.random.bernoulli(jax.random.key(42), KEEP, np.shape(x))
        )
    xd = np.where(keep, np.asarray(x, np.float32) / np.float32(KEEP), np.float32(0.0))
    return xd.astype(ml_dtypes.bfloat16)


def _prep_edges(edge_src, edge_dst, edge_w):
    """Group edges by (core, segment, bank); pad chunk counts to the
    cross-core max; emit per-core idx16 (wrapped), ew, rel arrays."""
    src = np.asarray(edge_src).astype(np.int64)
    dst = np.asarray(edge_dst).astype(np.int64)
    w = np.asarray(edge_w, np.float32)

    core = dst // RPC
    seg = (dst - core * RPC) // SEG
    bank = src // BANK
    gkey = (core * NSEG + seg) * 2 + bank
    order = np.argsort(gkey, kind="stable")
    src_s, dst_s, w_s, core_s, seg_s, bank_s = (
        src[order], dst[order], w[order], core[order], seg[order], bank[order]
    )
    rel_s = (dst_s - core_s * RPC - seg_s * SEG).astype(np.float32)
    srcloc_s = (src_s - bank_s * BANK).astype(np.int16)

    counts = np.bincount(gkey, minlength=N_CORES * NSEG * 2).reshape(
        N_CORES, NSEG * 2
    )
    C_sb = -(-counts // P)          # ceil
    C_sb = C_sb.max(axis=0)          # [NSEG*2] chunk counts padded across cores
    if C_sb.sum() == 0:
        C_sb[0] = 1
    tot_c = int(C_sb.sum())
    L = tot_c * P

    g_off = np.zeros(NSEG * 2 + 1, np.int64)
    g_off[1:] = np.cumsum(C_sb * P)
    e_off = np.zeros(N_CORES * NSEG * 2 + 1, np.int64)
    e_off[1:] = np.cumsum(counts.reshape(-1))

    idx_flat = np.zeros((N_CORES, L), np.int16)
    w_flat = np.zeros((N_CORES, L), np.float32)
    rel_flat = np.zeros((N_CORES, L), np.float32)
    for c in range(N_CORES):
        for g in range(NSEG * 2):
            k = c * NSEG * 2 + g
            n = int(counts[c, g])
            if n == 0:
                continue
            sl = slice(int(e_off[k]), int(e_off[k]) + n)
            d0 = int(g_off[g])
            idx_flat[c, d0 : d0 + n] = srcloc_s[sl]
            w_flat[c, d0 : d0 + n] = w_s[sl]
            rel_flat[c, d0 : d0 + n] = rel_s[sl]

    # wrapped int16 index layout for dma_gather: idx16[p, t] = flat[t*16 + p%16]
    idx16 = idx_flat.reshape(N_CORES, L // 16, 16).transpose(0, 2, 1)
    idx16 = np.tile(idx16, (1, 8, 1)).copy()                 # [NC, 128, L/16]
    ew = np.ascontiguousarray(w_flat.reshape(N_CORES, tot_c, P).transpose(0, 2, 1))
    rel = np.ascontiguousarray(rel_flat.reshape(N_CORES, tot_c, P).transpose(0, 2, 1))

    # per-core actual edge counts per emitted gather (same (s,b) order the
    # program emits: skipping groups with C_sb == 0), >= 1 so the gather
    # always emits at least one descriptor (sem-inc safety)
    cnts = []
    for c in range(N_CORES):
        row = []
        for g in range(NSEG * 2):
            if C_sb[g] == 0:
                continue
            row.append(max(1, int(counts[c, g])))
        cnts.append(row)
    cnt = np.asarray(cnts, np.int32).reshape(N_CORES, 1, -1)
    return C_sb.reshape(NSEG, 2), tot_c, idx16, ew, rel, cnt


def _build_program(c_key, tot_c):
    import concourse.bacc as bacc
    import concourse.mybir as mybir
    import concourse.tile as tile
    from contextlib import ExitStack

    C_sb = np.asarray(c_key, np.int64).reshape(NSEG, 2)
    dt = mybir.dt
    L16 = tot_c * P // 16
    n_gathers = int((C_sb > 0).sum())
    cb_max = int(C_sb.max())

    nc = bacc.Bacc("TRN2", target_bir_lowering=False, debug=False)
    xd_d = nc.dram_tensor("xd", [N_NODES, F], dt.bfloat16, kind="ExternalInput").ap()
    wm_d = nc.dram_tensor("wm", [F, F], dt.bfloat16, kind="ExternalInput").ap()
    bias_d = nc.dram_tensor("bias", [1, F], dt.bfloat16, kind="ExternalInput").ap()
    iota_d = nc.dram_tensor("iota", [P, P], dt.bfloat16, kind="ExternalInput").ap()
    ident_d = nc.dram_tensor("ident", [P, P], dt.bfloat16, kind="ExternalInput").ap()
    idx_d = nc.dram_tensor("idx16", [P, L16], dt.int16, kind="ExternalInput").ap()
    ew_d = nc.dram_tensor("ew", [P, tot_c], dt.float32, kind="ExternalInput").ap()
    rel_d = nc.dram_tensor("rel", [P, tot_c], dt.float32, kind="ExternalInput").ap()
    cnt_d = nc.dram_tensor("cnt", [1, n_gathers], dt.int32, kind="ExternalInput").ap()
    out_d = nc.dram_tensor("out", [RPC, F], dt.float32, kind="ExternalOutput").ap()

    with tile.TileContext(nc) as tc:
        with ExitStack() as ctx:
            const = ctx.enter_context(tc.tile_pool(name="const", bufs=1))
            gpool = ctx.enter_context(tc.tile_pool(name="gath", bufs=1))
            spool = ctx.enter_context(tc.tile_pool(name="smat", bufs=10))
            apool = ctx.enter_context(tc.tile_pool(name="acts", bufs=4))
            opool = ctx.enter_context(tc.tile_pool(name="outs", bufs=3))
            ppag = ctx.enter_context(tc.tile_pool(name="pag", bufs=3, space="PSUM"))
            pptr = ctx.enter_context(tc.tile_pool(name="ptr", bufs=2, space="PSUM"))
            ppout = ctx.enter_context(tc.tile_pool(name="pout", bufs=2, space="PSUM"))

            wm_sb = const.tile([P, 4, F], dt.bfloat16)
            nc.sync.dma_start(wm_sb[:], wm_d.rearrange("(k p) n -> p k n", p=P))
            iota_sb = const.tile([P, P], dt.bfloat16)
            nc.sync.dma_start(iota_sb[:], iota_d[:])
            ident_sb = const.tile([P, P], dt.bfloat16)
            nc.sync.dma_start(ident_sb[:], ident_d[:])
            b_sb = const.tile([1, F], dt.bfloat16)
            nc.sync.dma_start(b_sb[:], bias_d[:])
            ones_sb = const.tile([1, P], dt.bfloat16)
            nc.vector.memset(ones_sb[:], 1.0)
            idx_sb = const.tile([P, L16], dt.int16)
            nc.sync.dma_start(idx_sb[:], idx_d[:])
            ew_sb = const.tile([P, tot_c], dt.float32)
            nc.sync.dma_start(ew_sb[:], ew_d[:])
            rel_sb = const.tile([P, tot_c], dt.float32)
            nc.sync.dma_start(rel_sb[:], rel_d[:])
            cnt_sb = const.tile([1, n_gathers], dt.int32)
            nc.sync.dma_start(cnt_sb[:], cnt_d[:])

            # Manually rotated gather buffers, memset once so rows beyond the
            # per-core dynamic gather count hold finite values (pad columns
            # have w=0, so any finite residue contributes nothing).
            NGBUF = 6
            gbufs = [
                gpool.tile([P, cb_max, F], dt.bfloat16, name=f"gbuf{i}")
                for i in range(NGBUF)
            ]
            for gb in gbufs:
                nc.vector.memset(gb[:], 0.0)

            col = 0
            gidx = 0
            nseg_build = int(os.environ.get("KDBG_SEGS", NSEG))
            for s in range(nseg_build):
                cs = [int(C_sb[s, 0]), int(C_sb[s, 1])]
                ctot = cs[0] + cs[1]
                pag = ppag.tile([P, F], dt.float32, tag="pag", name="pag") if ctot else None
                jj = 0
                for bnk in (0, 1):
                    cb = cs[bnk]
                    if cb == 0:
                        continue
                    gt = gbufs[gidx % NGBUF]
                    nreg = cb * P
                    src_view = xd_d[0:BANK, :] if bnk == 0 else xd_d[BANK:N_NODES, :]
                    nc.gpsimd.dma_gather(
                        gt[:, :cb, :], src_view, idx_sb[:, col * 8 : (col + cb) * 8],
                        cb * P, nreg, F, single_packet=False,
                    )
                    gidx += 1
                    for j in range(cb):
                        smat = spool.tile([P, P], dt.bfloat16, tag="s", name="smat")
                        nc.vector.tensor_scalar(
                            out=smat[:],
                            in0=iota_sb[:],
                            scalar1=rel_sb[:, col + j : col + j + 1],
                            scalar2=ew_sb[:, col + j : col + j + 1],
                            op0=mybir.AluOpType.is_equal,
                            op1=mybir.AluOpType.mult,
                        )
                        nc.tensor.matmul(
                            pag[:], lhsT=smat[:], rhs=gt[:, j, :],
                            start=(jj == 0), stop=(jj == ctot - 1),
                        )
                        jj += 1
                    col += cb

                pout = ppout.tile([P, F], dt.float32, tag="pout", name="pout")
                if ctot:
                    agg_sb = apool.tile([P, F], dt.bfloat16, tag="agg", name="agg_sb")
                    nc.vector.tensor_copy(agg_sb[:], pag[:])
                    ptr = pptr.tile([P, F], dt.bfloat16, tag="ptr", name="ptr")
                    for k in range(4):
                        nc.tensor.transpose(
                            out=ptr[:, k * P : (k + 1) * P],
                            in_=agg_sb[:, k * P : (k + 1) * P],
                            identity=ident_sb[:],
                        )
                    aggt_sb = apool.tile([P, F], dt.bfloat16, tag="aggT", name="aggt_sb")
                    nc.scalar.copy(aggt_sb[:], ptr[:])
                    for k in range(4):
                        nc.tensor.matmul(
                            pout[:], lhsT=aggt_sb[:, k * P : (k + 1) * P],
                            rhs=wm_sb[:, k, :], start=(k == 0), stop=False,
                        )
                    nc.tensor.matmul(
                        pout[:], lhsT=ones_sb[:], rhs=b_sb[:], start=False, stop=True
                    )
                else:
                    nc.tensor.matmul(
                        pout[:], lhsT=ones_sb[:], rhs=b_sb[:], start=True, stop=True
                    )
                out_sb = opool.tile([P, F], dt.float32, tag="o", name="out_sb")
                nc.scalar.activation(
                    out_sb[:], pout[:], mybir.ActivationFunctionType.Relu
                )
                rows = SEG if s < NSEG - 1 else RPC - SEG * (NSEG - 1)
                nc.sync.dma_start(
                    out_d[s * SEG : s * SEG + rows, :], out_sb[:rows, :]
                )

    nc.compile()
    return nc


def kernel(x, edge_w, W, b, edge_src, edge_dst):
    global LAST_RESULTS
    _ensure_axon_ntff_hook()
    from concourse.bass_utils import run_bass_kernel_spmd

    xd16 = _dropout_table(x)
    C_sb, tot_c, idx16, ew, rel, cnt = _prep_edges(edge_src, edge_dst, edge_w)

    c_key = tuple(int(v) for v in C_sb.reshape(-1))
    cache_key = (c_key, tot_c)
    if cache_key not in _PROGRAM_CACHE:
        _PROGRAM_CACHE[cache_key] = _build_program(c_key, tot_c)
    nc = _PROGRAM_CACHE[cache_key]

    wm16 = np.asarray(W, np.float32).astype(ml_dtypes.bfloat16)
    b16 = np.asarray(b, np.float32).astype(ml_dtypes.bfloat16).reshape(1, F)
    iota = np.broadcast_to(
        np.arange(P, dtype=np.float32), (P, P)
    ).astype(ml_dtypes.bfloat16).copy()
    ident = np.eye(P, dtype=np.float32).astype(ml_dtypes.bfloat16)

    in_maps = []
    for c in range(N_CORES):
        in_maps.append({
            "xd": xd16,
            "wm": wm16,
            "bias": b16,
            "iota": iota,
            "ident": ident,
            "idx16": np.ascontiguousarray(idx16[c]),
            "ew": ew[c],
            "rel": rel[c],
            "cnt": cnt[c],
        })

    ncores_run = int(os.environ.get("KDBG_CORES", N_CORES))
    res = run_bass_kernel_spmd(nc, in_maps[:ncores_run], core_ids=list(range(ncores_run)))
    LAST_RESULTS = res
    outs = [res.results[c]["out"] for c in range(ncores_run)]
    outs += [np.zeros((RPC, F), np.float32)] * (N_CORES - ncores_run)
    out = np.concatenate(outs, axis=0)
    return np.ascontiguousarray(out.astype(np.float32))


# revision 28
# speedup vs baseline: 1.0495x; 1.0495x over previous
"""GCN layer (dropout -> weighted segment-sum aggregation -> linear -> ReLU)
on 8 Trainium2 NeuronCores.

Strategy (matches the "shard nodes by destination" hint):
  - Destination nodes (output rows) are sharded 6250/core across 8 cores.
  - Edges are partitioned by destination (edge_dst is sorted), then grouped
    into 128-destination segments per core; each segment's edges are chunked
    into groups of 128.
  - Source features are gathered per-edge from a replicated (dropout-applied,
    bf16) copy of x via the GPSIMD dma_gather custom op (int16 indices, so x
    is addressed through two 25000-row bank views).
  - Weighted segment-sum runs on the tensor engine: for each 128-edge chunk a
    selection matrix S[e, d] = w_e * (dst_e == d) is built on the vector
    engine (iota == rel) * w and matmul'd against the gathered rows,
    accumulating agg[d, f] for the segment in PSUM.
  - agg is transposed on the PE (via identity), multiplied by W (+ bias via a
    ones-row matmul), ReLU'd on the scalar engine, and DMA'd out.

All 8 cores run the same program (SPMD, one NEFF); per-core data differences
(edge partitions) are carried entirely by the input tensors. Per-(segment,
bank) chunk counts are padded to the max across cores with zero-weight edges.

The dropout mask depends only on the fixed key(42) and the input shape, so it
is constant-folded into the replicated feature table on the host.
"""

import os

import numpy as np
import ml_dtypes

# Problem constants (hardcoded per the harness contract).
N_NODES = 50000
N_EDGES = 800000
F = 512
P = 128
N_CORES = 8
RPC = N_NODES // N_CORES          # 6250 rows (dst nodes) per core
SEG = 128                          # dst nodes per segment
NSEG = (RPC + SEG - 1) // SEG      # 49 (last segment has 106 rows)
VIEW0_LEN = 32768                  # view 0 covers x rows [0, 32768)
VIEW1 = N_NODES - 32768            # view 1 covers x rows [17232, 50000)
KEEP = 0.9                         # 1 - dropout p

LAST_RESULTS = None                # BassKernelResults of the last run (for profiling)
_PROGRAM_CACHE = {}


def _ensure_axon_ntff_hook():
    """bass_utils imports antenv.axon_hooks when BASS_TRACE=1 under axon;
    some images lack that module. Provide it (with the ctypes NTFF hook from
    trn_agent_boot when available) so tracing works instead of crashing."""
    import sys
    import types

    try:
        import antenv.axon_hooks  # noqa: F401
        return
    except ImportError:
        pass
    try:
        import antenv
    except ImportError:
        return
    hook = None
    try:
        from trn_agent_boot.trn_boot import _ntff_profile_via_ctypes

        hook = _ntff_profile_via_ctypes("/opt/axon/libaxon_pjrt.so")
    except Exception:
        hook = None
    mod = types.ModuleType("antenv.axon_hooks")
    _state = {"hook": hook}
    mod.get_axon_ntff_profile_hook = lambda: _state["hook"]
    mod.set_axon_ntff_profile_hook = lambda h: _state.__setitem__("hook", h)
    sys.modules["antenv.axon_hooks"] = mod
    antenv.axon_hooks = mod


def _dropout_table(x):
    """Apply the deterministic dropout (jax key 42) and cast to bf16."""
    import jax

    cpu = jax.devices("cpu")[0]
    with jax.default_device(cpu):
        keep = np.asarray(
            jax.random.bernoulli(jax.random.key(42), KEEP, np.shape(x))
        )
    xd = np.where(keep, np.asarray(x, np.float32) / np.float32(KEEP), np.float32(0.0))
    return xd.astype(ml_dtypes.bfloat16)


def _prep_edges(edge_src, edge_dst, edge_w):
    """Group edges by (core, segment, gather-slot); the two gather slots read
    overlapping 32768-row views of x (V0 = rows [0, 32768), V1 = rows
    [VIEW1, 50000)), so edges with src in the overlap can go to either slot.
    That freedom balances the two slots' chunk counts per segment and removes
    most cross-core padding. Emits per-core idx16 (wrapped), ew, rel arrays."""
    src = np.asarray(edge_src).astype(np.int64)
    dst = np.asarray(edge_dst).astype(np.int64)
    w = np.asarray(edge_w, np.float32)

    core = dst // RPC
    seg = (dst - core * RPC) // SEG
    gkey = core * NSEG + seg
    order = np.argsort(gkey, kind="stable")
    src_s, w_s = src[order], w[order]
    rel_s = (dst[order] - core[order] * RPC - seg[order] * SEG).astype(np.float32)

    counts = np.bincount(gkey, minlength=N_CORES * NSEG).reshape(N_CORES, NSEG)
    e_off = np.zeros(N_CORES * NSEG + 1, np.int64)
    e_off[1:] = np.cumsum(counts.reshape(-1))

    # per-(core, segment) category counts
    n_lo = np.zeros((N_CORES, NSEG), np.int64)
    n_hi = np.zeros((N_CORES, NSEG), np.int64)
    lo_mask = src < VIEW1
    hi_mask = src >= VIEW0_LEN
    np.add.at(n_lo, (core[lo_mask], seg[lo_mask]), 1)
    np.add.at(n_hi, (core[hi_mask], seg[hi_mask]), 1)

    c_tot = (-(-counts // P)).max(axis=0)            # [NSEG]
    lo_min = (-(-n_lo // P)).max(axis=0)
    hi_min = (-(-n_hi // P)).max(axis=0)
    c_tot = np.maximum(c_tot, lo_min + hi_min)       # ensure feasibility
    c_tot = np.maximum(c_tot, 1)
    c0 = np.clip((lo_min + (c_tot - hi_min) + 1) // 2, lo_min, c_tot - hi_min)
    c1 = c_tot - c0
    C_sb = np.stack([c0, c1], axis=1)                # [NSEG, 2]
    tot_c = int(C_sb.sum())
    L = tot_c * P

    g_off = np.zeros(NSEG * 2 + 1, np.int64)
    g_off[1:] = np.cumsum(C_sb.reshape(-1) * P)

    idx_flat = np.zeros((N_CORES, L), np.int16)
    w_flat = np.zeros((N_CORES, L), np.float32)
    rel_flat = np.zeros((N_CORES, L), np.float32)
    for c in range(N_CORES):
        for s in range(NSEG):
            k = c * NSEG + s
            n = int(counts[c, s])
            if n == 0:
                continue
            sl = slice(int(e_off[k]), int(e_off[k]) + n)
            e_src, e_w, e_rel = src_s[sl], w_s[sl], rel_s[sl]
            cap0 = int(C_sb[s, 0]) * P
            # slot0: all lo edges plus enough mid edges (by ascending src)
            in0 = e_src < VIEW1
            n_mid_to0 = cap0 - int(in0.sum())
            if n_mid_to0 > 0:
                mid_idx = np.where((~in0) & (e_src < VIEW0_LEN))[0]
                if len(mid_idx) > n_mid_to0:
                    mid_idx = mid_idx[:n_mid_to0]
                in0[mid_idx] = True
            for slot in (0, 1):
                m = in0 if slot == 0 else ~in0
                nn = int(m.sum())
                if nn == 0:
                    continue
                base = 0 if slot == 0 else VIEW1
                d0 = int(g_off[2 * s + slot])
                idx_flat[c, d0 : d0 + nn] = (e_src[m] - base).astype(np.int16)
                w_flat[c, d0 : d0 + nn] = e_w[m]
                rel_flat[c, d0 : d0 + nn] = e_rel[m]

    # wrapped int16 index layout for dma_gather: idx16[p, t] = flat[t*16 + p%16]
    idx16 = idx_flat.reshape(N_CORES, L // 16, 16).transpose(0, 2, 1)
    idx16 = np.tile(idx16, (1, 8, 1)).copy()                 # [NC, 128, L/16]
    ew = np.ascontiguousarray(w_flat.reshape(N_CORES, tot_c, P).transpose(0, 2, 1))
    rel = np.ascontiguousarray(rel_flat.reshape(N_CORES, tot_c, P).transpose(0, 2, 1))
    return C_sb, tot_c, idx16, ew, rel


def _build_program(c_key, tot_c):
    import concourse.bacc as bacc
    import concourse.mybir as mybir
    import concourse.tile as tile
    from contextlib import ExitStack

    C_sb = np.asarray(c_key, np.int64).reshape(NSEG, 2)
    dt = mybir.dt
    L16 = tot_c * P // 16
    n_gathers = int((C_sb > 0).sum())
    cb_max = int(C_sb.max())

    nc = bacc.Bacc("TRN2", target_bir_lowering=False, debug=False)
    xd_d = nc.dram_tensor("xd", [N_NODES, F], dt.bfloat16, kind="ExternalInput").ap()
    wm_d = nc.dram_tensor("wm", [F, F], dt.bfloat16, kind="ExternalInput").ap()
    bias_d = nc.dram_tensor("bias", [1, F], dt.bfloat16, kind="ExternalInput").ap()
    iota_d = nc.dram_tensor("iota", [P, P], dt.bfloat16, kind="ExternalInput").ap()
    ident_d = nc.dram_tensor("ident", [P, P], dt.bfloat16, kind="ExternalInput").ap()
    idx_d = nc.dram_tensor("idx16", [P, L16], dt.int16, kind="ExternalInput").ap()
    ew_d = nc.dram_tensor("ew", [P, tot_c], dt.float32, kind="ExternalInput").ap()
    rel_d = nc.dram_tensor("rel", [P, tot_c], dt.float32, kind="ExternalInput").ap()
    out_d = nc.dram_tensor("out", [RPC, F], dt.float32, kind="ExternalOutput").ap()

    with tile.TileContext(nc) as tc:
        with ExitStack() as ctx:
            const = ctx.enter_context(tc.tile_pool(name="const", bufs=1))
            gpool = ctx.enter_context(tc.tile_pool(name="gath", bufs=1))
            spool = ctx.enter_context(tc.tile_pool(name="smat", bufs=10))
            apool = ctx.enter_context(tc.tile_pool(name="acts", bufs=4))
            opool = ctx.enter_context(tc.tile_pool(name="outs", bufs=3))
            ppag = ctx.enter_context(tc.tile_pool(name="pag", bufs=3, space="PSUM"))
            pptr = ctx.enter_context(tc.tile_pool(name="ptr", bufs=2, space="PSUM"))
            ppout = ctx.enter_context(tc.tile_pool(name="pout", bufs=2, space="PSUM"))

            wm_sb = const.tile([P, 4, F], dt.bfloat16)
            nc.sync.dma_start(wm_sb[:], wm_d.rearrange("(k p) n -> p k n", p=P))
            iota_sb = const.tile([P, P], dt.bfloat16)
            nc.sync.dma_start(iota_sb[:], iota_d[:])
            ident_sb = const.tile([P, P], dt.bfloat16)
            nc.sync.dma_start(ident_sb[:], ident_d[:])
            b_sb = const.tile([1, F], dt.bfloat16)
            nc.sync.dma_start(b_sb[:], bias_d[:])
            ones_sb = const.tile([1, P], dt.bfloat16)
            nc.vector.memset(ones_sb[:], 1.0)
            idx_sb = const.tile([P, L16], dt.int16)
            nc.sync.dma_start(idx_sb[:], idx_d[:])
            ew_sb = const.tile([P, tot_c], dt.float32)
            nc.sync.dma_start(ew_sb[:], ew_d[:])
            rel_sb = const.tile([P, tot_c], dt.float32)
            nc.sync.dma_start(rel_sb[:], rel_d[:])

            pfill = ctx.enter_context(tc.tile_pool(name="pfill", bufs=1, space="PSUM"))
            fill_in = const.tile([P, 256], dt.bfloat16)
            nc.vector.memset(fill_in[:], 1.0 / 64.0)
            fill_ps = pfill.tile([64, 256], dt.float32, name="fill_ps")
            nfill = 0 if os.environ.get("KDBG_NOFILL") else 36

            # Manually rotated gather buffers, memset once so rows beyond the
            # per-core dynamic gather count hold finite values (pad columns
            # have w=0, so any finite residue contributes nothing).
            NGBUF = 6
            gbufs = [
                gpool.tile([P, cb_max, F], dt.bfloat16, name=f"gbuf{i}")
                for i in range(NGBUF)
            ]
            for gb in gbufs:
                nc.vector.memset(gb[:], 0.0)

            col = 0
            gidx = 0
            nseg_build = int(os.environ.get("KDBG_SEGS", NSEG))
            for s in range(nseg_build):
                cs = [int(C_sb[s, 0]), int(C_sb[s, 1])]
                ctot = cs[0] + cs[1]
                pag = ppag.tile([P, F], dt.float32, tag="pag", name="pag") if ctot else None
                jj = 0
                for bnk in (0, 1):
                    cb = cs[bnk]
                    if cb == 0:
                        continue
                    gt = gbufs[gidx % NGBUF]
                    nreg = cb * P
                    src_view = (
                        xd_d[0:VIEW0_LEN, :] if bnk == 0 else xd_d[VIEW1:N_NODES, :]
                    )
                    nc.gpsimd.dma_gather(
                        gt[:, :cb, :], src_view, idx_sb[:, col * 8 : (col + cb) * 8],
                        cb * P, nreg, F, single_packet=False,
                    )
                    gidx += 1
                    for j in range(cb):
                        smat = spool.tile([P, P], dt.bfloat16, tag="s", name="smat")
                        nc.vector.tensor_scalar(
                            out=smat[:],
                            in0=iota_sb[:],
                            scalar1=rel_sb[:, col + j : col + j + 1],
                            scalar2=ew_sb[:, col + j : col + j + 1],
                            op0=mybir.AluOpType.is_equal,
                            op1=mybir.AluOpType.mult,
                        )
                        nc.tensor.matmul(
                            pag[:], lhsT=smat[:], rhs=gt[:, j, :],
                            start=(jj == 0), stop=(jj == ctot - 1),
                        )
                        jj += 1
                    col += cb

                pout = ppout.tile([P, F], dt.float32, tag="pout", name="pout")
                if ctot:
                    agg_sb = apool.tile([P, F], dt.bfloat16, tag="agg", name="agg_sb")
                    nc.vector.tensor_copy(agg_sb[:], pag[:])
                    ptr = pptr.tile([P, F], dt.bfloat16, tag="ptr", name="ptr")
                    for k in range(4):
                        nc.tensor.transpose(
                            out=ptr[:, k * P : (k + 1) * P],
                            in_=agg_sb[:, k * P : (k + 1) * P],
                            identity=ident_sb[:],
                        )
                    aggt_sb = apool.tile([P, F], dt.bfloat16, tag="aggT", name="aggt_sb")
                    nc.scalar.copy(aggt_sb[:], ptr[:])
                    for k in range(4):
                        nc.tensor.matmul(
                            pout[:], lhsT=aggt_sb[:, k * P : (k + 1) * P],
                            rhs=wm_sb[:, k, :], start=(k == 0), stop=False,
                        )
                    nc.tensor.matmul(
                        pout[:], lhsT=ones_sb[:], rhs=b_sb[:], start=False, stop=True
                    )
                else:
                    nc.tensor.matmul(
                        pout[:], lhsT=ones_sb[:], rhs=b_sb[:], start=True, stop=True
                    )
                out_sb = opool.tile([P, F], dt.float32, tag="o", name="out_sb")
                nc.scalar.activation(
                    out_sb[:], pout[:], mybir.ActivationFunctionType.Relu
                )
                rows = SEG if s < NSEG - 1 else RPC - SEG * (NSEG - 1)
                nc.sync.dma_start(
                    out_d[s * SEG : s * SEG + rows, :], out_sb[:rows, :]
                )

                # HAM-warming filler: keeps the PE busy while the next
                # segment's gather is still generating descriptors, so real
                # matmuls run at 2.4 GHz instead of the cold 1.2 GHz.
                if nfill and s < nseg_build - 1:
                    for fi in range(nfill):
                        nc.tensor.matmul(
                            fill_ps[:], lhsT=fill_in[:, :64], rhs=fill_in[:],
                            start=(fi == 0), stop=(fi == nfill - 1),
                        )

    nc.compile()
    return nc


def kernel(x, edge_w, W, b, edge_src, edge_dst):
    global LAST_RESULTS
    _ensure_axon_ntff_hook()
    from concourse.bass_utils import run_bass_kernel_spmd

    xd16 = _dropout_table(x)
    C_sb, tot_c, idx16, ew, rel = _prep_edges(edge_src, edge_dst, edge_w)

    c_key = tuple(int(v) for v in C_sb.reshape(-1))
    cache_key = (c_key, tot_c)
    if cache_key not in _PROGRAM_CACHE:
        _PROGRAM_CACHE[cache_key] = _build_program(c_key, tot_c)
    nc = _PROGRAM_CACHE[cache_key]

    wm16 = np.asarray(W, np.float32).astype(ml_dtypes.bfloat16)
    b16 = np.asarray(b, np.float32).astype(ml_dtypes.bfloat16).reshape(1, F)
    iota = np.broadcast_to(
        np.arange(P, dtype=np.float32), (P, P)
    ).astype(ml_dtypes.bfloat16).copy()
    ident = np.eye(P, dtype=np.float32).astype(ml_dtypes.bfloat16)

    in_maps = []
    for c in range(N_CORES):
        in_maps.append({
            "xd": xd16,
            "wm": wm16,
            "bias": b16,
            "iota": iota,
            "ident": ident,
            "idx16": np.ascontiguousarray(idx16[c]),
            "ew": ew[c],
            "rel": rel[c],
        })

    ncores_run = int(os.environ.get("KDBG_CORES", N_CORES))
    res = run_bass_kernel_spmd(nc, in_maps[:ncores_run], core_ids=list(range(ncores_run)))
    LAST_RESULTS = res
    outs = [res.results[c]["out"] for c in range(ncores_run)]
    outs += [np.zeros((RPC, F), np.float32)] * (N_CORES - ncores_run)
    out = np.concatenate(outs, axis=0)
    return np.ascontiguousarray(out.astype(np.float32))
